# revision 7
# baseline (speedup 1.0000x reference)
"""DCTFreqConv Trainium2 kernel: 8x8-block DCT2 -> Conv1d over 64 freqs
(64ch mix, win 3, causal-right pad) -> IDCT2. Data-parallel: 1 batch
sample per NeuronCore (8 cores).

Pipeline per core (all matmuls on PE, fp32):
  S1  DCT-h + transpose    (x-tile as lhsT, A^T as rhs)  -> [w | (c,kh)]
  S2  DCT-w                (A^T as lhsT)                 -> [kw | (c,kh)]
  S3  promote channels     (rhs = I128)                  -> [ci | kw] per kh
  S4  conv: 3 accumulating matmuls over f-shifted views  -> [co | (wb,f)]
  S5  demote channels      (rhs = I64, per (hb,fh,wT))   -> [kw | co]
  S6  IDCT-w + promote kh  (buf5 as lhsT, A as rhs)      -> [kh | w]
  S7  IDCT-h               (A as lhsT)                   -> [h | (co,w)] -> HBM
where A = I16 (x) D (128x128 block-diagonal DCT), per 128-half of each axis.
"""
import numpy as np

N_CORES = 8
C = 64
H = W = 256
B = 8

_nc_cache = {}


def _dct_mat():
    n = np.arange(B)
    k = n[:, None]
    D = np.sqrt(2.0 / B) * np.cos(np.pi * (2 * n[None, :] + 1) * k / (2 * B))
    D[0, :] *= 1.0 / np.sqrt(2.0)
    return D.astype(np.float32)


def _build():
    import concourse.bacc as bacc
    import concourse.mybir as mybir
    import concourse.tile as tile

    f32 = mybir.dt.float32
    nc = bacc.Bacc("TRN2", target_bir_lowering=False)

    x_d = nc.dram_tensor("x", (C, H, W), f32, kind="ExternalInput")
    cAT_d = nc.dram_tensor("cAT", (128, 128), f32, kind="ExternalInput")
    cA_d = nc.dram_tensor("cA", (128, 128), f32, kind="ExternalInput")
    cI128_d = nc.dram_tensor("cI128", (128, 128), f32, kind="ExternalInput")
    cI64_d = nc.dram_tensor("cI64d", (128, 64), f32, kind="ExternalInput")
    cW_d = nc.dram_tensor("cW", (3, 128, 64), f32, kind="ExternalInput")
    cB_d = nc.dram_tensor("cBd", (128, 1), f32, kind="ExternalInput")
    out_d = nc.dram_tensor("out", (C, H, W), f32, kind="ExternalOutput")

    Copy = mybir.ActivationFunctionType.Identity

    with tile.TileContext(nc) as tc:
        with (
            tc.tile_pool(name="consts", bufs=1) as cpool,
            tc.tile_pool(name="xin", bufs=4) as xpool,
            tc.tile_pool(name="big", bufs=1) as bigpool,
            tc.tile_pool(name="ring", bufs=1) as ringpool,
            tc.tile_pool(name="outp", bufs=4) as opool,
            tc.tile_pool(name="ps", bufs=6, space="PSUM") as pspool,
        ):
            cAT = cpool.tile([128, 128], f32)
            nc.sync.dma_start(out=cAT, in_=cAT_d[:, :])
            cA = cpool.tile([128, 128], f32)
            nc.sync.dma_start(out=cA, in_=cA_d[:, :])
            cI128 = cpool.tile([128, 128], f32)
            nc.sync.dma_start(out=cI128, in_=cI128_d[:, :])
            cI64 = cpool.tile([128, 64], f32)
            nc.sync.dma_start(out=cI64, in_=cI64_d[:, :])
            cW = cpool.tile([128, 3, 64], f32)
            nc.sync.dma_start(out=cW, in_=cW_d[:, :, :].rearrange("d p c -> p d c"))
            cB = cpool.tile([128, 1], f32)
            nc.sync.dma_start(out=cB, in_=cB_d[:, :])

            for hH in range(2):
                hsl = slice(hH * 128, (hH + 1) * 128)
                # buf2[wT]: [kw | (c, kh_local)]
                buf2 = [
                    bigpool.tile([128, C, 128], f32, name=f"buf2_{hH}_{w}", tag="buf2", bufs=2)
                    for w in range(2)
                ]
                # buf5[wT]: [kw | (kh_local, co)]
                buf5 = [
                    bigpool.tile([128, 128, C], f32, name=f"buf5_{hH}_{w}", tag="buf15", bufs=2)
                    for w in range(2)
                ]
                # ---- S1: DCT-h + transpose ----
                buf1 = [
                    bigpool.tile([128, C, 128], f32, name=f"buf1_{hH}_{w}",
                                 tag="buf15", bufs=2)
                    for w in range(2)
                ]
                for c in range(C):
                    xt = xpool.tile([128, 256], f32, name=f"xt_{hH}_{c}", tag="xt")
                    nc.sync.dma_start(out=xt, in_=x_d[c, hsl, :])
                    for wT in range(2):
                        ps1 = pspool.tile([128, 512], f32, name="ps1", tag="ps")
                        nc.tensor.matmul(
                            out=ps1[:, 0:128],
                            lhsT=xt[:, wT * 128:(wT + 1) * 128],
                            rhs=cAT,
                        )
                        nc.any.tensor_copy(out=buf1[wT][:, c, :], in_=ps1[:, 0:128])
                # ---- S2: DCT-w ----
                for wT in range(2):
                    for cg in range(C // 4):
                        ps2 = pspool.tile([128, 512], f32, name="ps2", tag="ps")
                        nc.tensor.matmul(
                            out=ps2[:, 0:512],
                            lhsT=cAT,
                            rhs=buf1[wT][:, cg * 4:(cg + 1) * 4, :],
                        )
                        nc.any.tensor_copy(
                            out=buf2[wT][:, cg * 4:(cg + 1) * 4, :],
                            in_=ps2[:, 0:512],
                        )

                # ---- hb-pair loop: S3 (promote c), S4 (conv), S5 (demote) ----
                for pr in range(8):  # hb pairs within this hH
                    buf3 = ringpool.tile([128, 32, 66], f32, name=f"b3_{hH}_{pr}",
                                         tag="buf3")
                    nc.vector.memset(buf3[:, :, 64:66], 0.0)
                    for r in range(2):  # hb parity within pair
                        hb_l = pr * 2 + r
                        for fh in range(8):
                            kh = hb_l * 8 + fh
                            for wT in range(2):
                                ps3 = pspool.tile([128, 512], f32, name="ps3",
                                                  tag="ps")
                                nc.tensor.matmul(
                                    out=ps3[r * 64:(r + 1) * 64, 0:128],
                                    lhsT=buf2[wT][:, :, kh],
                                    rhs=cI128,
                                )
                                # scatter [ci | kw=(wb16, fw8)] into padded layout
                                nc.any.tensor_copy(
                                    out=buf3[r * 64:(r + 1) * 64,
                                             wT * 16:(wT + 1) * 16,
                                             fh * 8:fh * 8 + 8],
                                    in_=ps3[r * 64:(r + 1) * 64, 0:128].rearrange(
                                        "p (wb fw) -> p wb fw", fw=8),
                                )
                    # buf4: [co | (fh, wb, fw)] so S5's lhsT slice is 1-D
                    buf4 = ringpool.tile([128, 8, 32, 8], f32, name=f"b4_{hH}_{pr}",
                                         tag="buf4")
                    for g in range(4):  # wb groups of 8
                        ps4 = pspool.tile([128, 512], f32, name="ps4", tag="ps")
                        for r in range(2):
                            for d in range(3):
                                nc.tensor.matmul(
                                    out=ps4[r * 64:(r + 1) * 64, 0:512],
                                    lhsT=cW[r * 64:(r + 1) * 64, d, :],
                                    rhs=buf3[r * 64:(r + 1) * 64,
                                             g * 8:(g + 1) * 8,
                                             d:d + 64],
                                    start=(d == 0),
                                    stop=(d == 2),
                                )
                        nc.scalar.activation(
                            out=buf4[:, :, g * 8:(g + 1) * 8, :].rearrange(
                                "p a b c -> p b a c"),
                            in_=ps4[:, 0:512],
                            func=Copy,
                            bias=cB[:, 0:1],
                        )
                    # ---- S5: demote channels ----
                    for r in range(2):
                        hb_l = pr * 2 + r
                        for fh in range(8):
                            khr = hb_l * 8 + fh
                            for wT in range(2):
                                ps5 = pspool.tile([128, 512], f32, name="ps5",
                                                  tag="ps")
                                nc.tensor.matmul(
                                    out=ps5[:, 0:64],
                                    lhsT=buf4[r * 64:(r + 1) * 64, fh,
                                              wT * 16:(wT + 1) * 16,
                                              :].rearrange("p w f -> p (w f)"),
                                    rhs=cI64[r * 64:(r + 1) * 64, :],
                                )
                                nc.any.tensor_copy(
                                    out=buf5[wT][:, khr, :], in_=ps5[:, 0:64])

                # ---- S6: IDCT-w + promote kh;  S7: IDCT-h; DMA out ----
                for cg in range(C // 4):
                    buf6 = ringpool.tile([128, 4, 256], f32, name=f"b6_{hH}_{cg}",
                                         tag="buf6")
                    for ci in range(4):
                        co = cg * 4 + ci
                        for wT in range(2):
                            ps6 = pspool.tile([128, 512], f32, name="ps6", tag="ps")
                            nc.tensor.matmul(
                                out=ps6[:, 0:128],
                                lhsT=buf5[wT][:, :, co],
                                rhs=cA,
                            )
                            nc.any.tensor_copy(
                                out=buf6[:, ci, wT * 128:(wT + 1) * 128],
                                in_=ps6[:, 0:128],
                            )
                    for p in range(2):  # co pairs
                        ps7 = pspool.tile([128, 512], f32, name="ps7", tag="ps")
                        nc.tensor.matmul(
                            out=ps7[:, 0:512],
                            lhsT=cA,
                            rhs=buf6[:, p * 2:(p + 1) * 2, :],
                        )
                        osb = opool.tile([128, 2, 256], f32, name="osb", tag="osb")
                        nc.any.tensor_copy(out=osb, in_=ps7[:, 0:512].rearrange(
                            "p (a b) -> p a b", a=2))
                        c0 = cg * 4 + p * 2
                        nc.sync.dma_start(
                            out=out_d[c0:c0 + 2, hsl, :].rearrange(
                                "c h w -> h c w"),
                            in_=osb,
                        )
    nc.finalize()
    return nc


def kernel(x, conv_w, conv_b):
    from concourse import bass_utils

    x = np.ascontiguousarray(np.asarray(x, dtype=np.float32))
    conv_w = np.asarray(conv_w, dtype=np.float32)
    conv_b = np.asarray(conv_b, dtype=np.float32)
    bsz = x.shape[0]
    assert x.shape == (8, C, H, W)

    if "nc" not in _nc_cache:
        _nc_cache["nc"] = _build()
    nc = _nc_cache["nc"]

    D = _dct_mat()
    A = np.kron(np.eye(16, dtype=np.float32), D).astype(np.float32)
    I64 = np.eye(64, dtype=np.float32)
    cW = np.stack(
        [np.vstack([conv_w[:, :, d].T, conv_w[:, :, d].T]) for d in range(3)]
    ).astype(np.float32)  # (3, 128, 64): [d][ci(dup), co]
    consts = {
        "cAT": np.ascontiguousarray(A.T),
        "cA": np.ascontiguousarray(A),
        "cI128": np.eye(128, dtype=np.float32),
        "cI64d": np.ascontiguousarray(np.vstack([I64, I64])),
        "cW": np.ascontiguousarray(cW),
        "cBd": np.ascontiguousarray(
            np.concatenate([conv_b, conv_b]).reshape(128, 1)),
    }
    in_maps = [{"x": np.ascontiguousarray(x[i]), **consts} for i in range(bsz)]
    res = bass_utils.run_bass_kernel_spmd(nc, in_maps, core_ids=list(range(N_CORES)))
    out = np.stack([res.results[i]["out"] for i in range(bsz)])
    return out.astype(np.float32)


# revision 8
# speedup vs baseline: 1.1141x; 1.1141x over previous
"""DCTFreqConv Trainium2 kernel: 8x8-block DCT2 -> Conv1d over 64 freqs
(64ch mix, win 3, causal-right pad) -> IDCT2. Data-parallel: 1 batch
sample per NeuronCore (8 cores).

Pipeline per core (all matmuls on PE, fp32):
  S1  DCT-h + transpose    (x-tile as lhsT, A^T as rhs)  -> [w | (c,kh)]
  S2  DCT-w                (A^T as lhsT)                 -> [kw | (c,kh)]
  S3  promote channels     (rhs = I128)                  -> [ci | kw] per kh
  S4  conv: 3 accumulating matmuls over f-shifted views  -> [co | (wb,f)]
  S5  demote channels      (rhs = I64, per (hb,fh,wT))   -> [kw | co]
  S6  IDCT-w + promote kh  (buf5 as lhsT, A as rhs)      -> [kh | w]
  S7  IDCT-h               (A as lhsT)                   -> [h | (co,w)] -> HBM
where A = I16 (x) D (128x128 block-diagonal DCT), per 128-half of each axis.
"""
import numpy as np

N_CORES = 8
C = 64
H = W = 256
B = 8

_nc_cache = {}


def _dct_mat():
    n = np.arange(B)
    k = n[:, None]
    D = np.sqrt(2.0 / B) * np.cos(np.pi * (2 * n[None, :] + 1) * k / (2 * B))
    D[0, :] *= 1.0 / np.sqrt(2.0)
    return D.astype(np.float32)


def _build():
    import concourse.bacc as bacc
    import concourse.mybir as mybir
    import concourse.tile as tile

    f32 = mybir.dt.float32
    nc = bacc.Bacc("TRN2", target_bir_lowering=False)

    x_d = nc.dram_tensor("x", (C, H, W), f32, kind="ExternalInput")
    cAT_d = nc.dram_tensor("cAT", (128, 128), f32, kind="ExternalInput")
    cA_d = nc.dram_tensor("cA", (128, 128), f32, kind="ExternalInput")
    cI128_d = nc.dram_tensor("cI128", (128, 128), f32, kind="ExternalInput")
    cI64_d = nc.dram_tensor("cI64d", (128, 64), f32, kind="ExternalInput")
    cW_d = nc.dram_tensor("cW", (3, 128, 64), f32, kind="ExternalInput")
    cB_d = nc.dram_tensor("cBd", (128, 1), f32, kind="ExternalInput")
    out_d = nc.dram_tensor("out", (C, H, W), f32, kind="ExternalOutput")

    Copy = mybir.ActivationFunctionType.Identity

    with tile.TileContext(nc) as tc:
        with (
            tc.tile_pool(name="consts", bufs=1) as cpool,
            tc.tile_pool(name="xin", bufs=8) as xpool,
            tc.tile_pool(name="big", bufs=1) as bigpool,
            tc.tile_pool(name="ring", bufs=1) as ringpool,
            tc.tile_pool(name="outp", bufs=4) as opool,
            tc.tile_pool(name="ps", bufs=8, space="PSUM") as pspool,
        ):
            cAT = cpool.tile([128, 128], f32)
            nc.sync.dma_start(out=cAT, in_=cAT_d[:, :])
            cA = cpool.tile([128, 128], f32)
            nc.sync.dma_start(out=cA, in_=cA_d[:, :])
            cI128 = cpool.tile([128, 128], f32)
            nc.sync.dma_start(out=cI128, in_=cI128_d[:, :])
            cI64 = cpool.tile([128, 64], f32)
            nc.sync.dma_start(out=cI64, in_=cI64_d[:, :])
            cW = cpool.tile([128, 3, 64], f32)
            nc.sync.dma_start(out=cW, in_=cW_d[:, :, :].rearrange("d p c -> p d c"))
            cB = cpool.tile([128, 1], f32)
            nc.sync.dma_start(out=cB, in_=cB_d[:, :])

            for hH in range(2):
                hsl = slice(hH * 128, (hH + 1) * 128)
                # buf2[wT]: [kw | (c, kh_local)]
                buf2 = [
                    bigpool.tile([128, C, 128], f32, name=f"buf2_{hH}_{w}", tag="buf2", bufs=2)
                    for w in range(2)
                ]
                # buf5[wT]: [kw | (kh_local, co)]
                buf5 = [
                    bigpool.tile([128, 128, C], f32, name=f"buf5_{hH}_{w}", tag="buf15", bufs=2)
                    for w in range(2)
                ]
                # ---- S1: DCT-h + transpose ----
                buf1 = [
                    bigpool.tile([128, C, 128], f32, name=f"buf1_{hH}_{w}",
                                 tag="buf15", bufs=2)
                    for w in range(2)
                ]
                for c in range(C):
                    xt = xpool.tile([128, 256], f32, name=f"xt_{hH}_{c}", tag="xt")
                    nc.sync.dma_start(out=xt, in_=x_d[c, hsl, :])
                    for wT in range(2):
                        ps1 = pspool.tile([128, 512], f32, name="ps1", tag="ps")
                        nc.tensor.matmul(
                            out=ps1[:, 0:128],
                            lhsT=xt[:, wT * 128:(wT + 1) * 128],
                            rhs=cAT,
                        )
                        nc.any.tensor_copy(out=buf1[wT][:, c, :], in_=ps1[:, 0:128])
                # ---- S2: DCT-w ----
                for wT in range(2):
                    for cg in range(C // 4):
                        ps2 = pspool.tile([128, 512], f32, name="ps2", tag="ps")
                        nc.tensor.matmul(
                            out=ps2[:, 0:512],
                            lhsT=cAT,
                            rhs=buf1[wT][:, cg * 4:(cg + 1) * 4, :],
                        )
                        nc.any.tensor_copy(
                            out=buf2[wT][:, cg * 4:(cg + 1) * 4, :],
                            in_=ps2[:, 0:512],
                        )

                # ---- hb-pair loop: S3 (promote c), S4 (conv), S5 (demote) ----
                for pr in range(8):  # hb pairs within this hH
                    buf3 = ringpool.tile([128, 32, 66], f32, name=f"b3_{hH}_{pr}",
                                         tag="buf3", bufs=2)
                    nc.vector.memset(buf3[:, :, 64:66], 0.0)
                    for r in range(2):  # hb parity within pair
                        hb_l = pr * 2 + r
                        for fh in range(8):
                            kh = hb_l * 8 + fh
                            for wT in range(2):
                                ps3 = pspool.tile([128, 512], f32, name="ps3",
                                                  tag="ps")
                                nc.tensor.matmul(
                                    out=ps3[r * 64:(r + 1) * 64, 0:128],
                                    lhsT=buf2[wT][:, :, kh],
                                    rhs=cI128,
                                )
                                # scatter [ci | kw=(wb16, fw8)] into padded layout
                                nc.any.tensor_copy(
                                    out=buf3[r * 64:(r + 1) * 64,
                                             wT * 16:(wT + 1) * 16,
                                             fh * 8:fh * 8 + 8],
                                    in_=ps3[r * 64:(r + 1) * 64, 0:128].rearrange(
                                        "p (wb fw) -> p wb fw", fw=8),
                                )
                    # buf4: [co | (fh, wb, fw)] so S5's lhsT slice is 1-D
                    buf4 = ringpool.tile([128, 8, 32, 8], f32, name=f"b4_{hH}_{pr}",
                                         tag="buf4", bufs=2)
                    for g in range(4):  # wb groups of 8
                        ps4 = pspool.tile([128, 512], f32, name="ps4", tag="ps")
                        for r in range(2):
                            for d in range(3):
                                nc.tensor.matmul(
                                    out=ps4[r * 64:(r + 1) * 64, 0:512],
                                    lhsT=cW[r * 64:(r + 1) * 64, d, :],
                                    rhs=buf3[r * 64:(r + 1) * 64,
                                             g * 8:(g + 1) * 8,
                                             d:d + 64],
                                    start=(d == 0),
                                    stop=(d == 2),
                                )
                        nc.scalar.activation(
                            out=buf4[:, :, g * 8:(g + 1) * 8, :].rearrange(
                                "p a b c -> p b a c"),
                            in_=ps4[:, 0:512],
                            func=Copy,
                            bias=cB[:, 0:1],
                        )
                    # ---- S5: demote channels ----
                    for r in range(2):
                        hb_l = pr * 2 + r
                        for fh in range(8):
                            khr = hb_l * 8 + fh
                            for wT in range(2):
                                ps5 = pspool.tile([128, 512], f32, name="ps5",
                                                  tag="ps")
                                nc.tensor.matmul(
                                    out=ps5[:, 0:64],
                                    lhsT=buf4[r * 64:(r + 1) * 64, fh,
                                              wT * 16:(wT + 1) * 16,
                                              :].rearrange("p w f -> p (w f)"),
                                    rhs=cI64[r * 64:(r + 1) * 64, :],
                                )
                                nc.any.tensor_copy(
                                    out=buf5[wT][:, khr, :], in_=ps5[:, 0:64])

                # ---- S6: IDCT-w + promote kh;  S7: IDCT-h; DMA out ----
                for cg in range(C // 4):
                    buf6 = ringpool.tile([128, 4, 256], f32, name=f"b6_{hH}_{cg}",
                                         tag="buf6", bufs=2)
                    for ci in range(4):
                        co = cg * 4 + ci
                        for wT in range(2):
                            ps6 = pspool.tile([128, 512], f32, name="ps6", tag="ps")
                            nc.tensor.matmul(
                                out=ps6[:, 0:128],
                                lhsT=buf5[wT][:, :, co],
                                rhs=cA,
                            )
                            nc.any.tensor_copy(
                                out=buf6[:, ci, wT * 128:(wT + 1) * 128],
                                in_=ps6[:, 0:128],
                            )
                    for p in range(2):  # co pairs
                        ps7 = pspool.tile([128, 512], f32, name="ps7", tag="ps")
                        nc.tensor.matmul(
                            out=ps7[:, 0:512],
                            lhsT=cA,
                            rhs=buf6[:, p * 2:(p + 1) * 2, :],
                        )
                        osb = opool.tile([128, 2, 256], f32, name="osb", tag="osb")
                        nc.any.tensor_copy(out=osb, in_=ps7[:, 0:512].rearrange(
                            "p (a b) -> p a b", a=2))
                        c0 = cg * 4 + p * 2
                        nc.sync.dma_start(
                            out=out_d[c0:c0 + 2, hsl, :].rearrange(
                                "c h w -> h c w"),
                            in_=osb,
                        )
    nc.finalize()
    return nc


def kernel(x, conv_w, conv_b):
    from concourse import bass_utils

    x = np.ascontiguousarray(np.asarray(x, dtype=np.float32))
    conv_w = np.asarray(conv_w, dtype=np.float32)
    conv_b = np.asarray(conv_b, dtype=np.float32)
    bsz = x.shape[0]
    assert x.shape == (8, C, H, W)

    if "nc" not in _nc_cache:
        _nc_cache["nc"] = _build()
    nc = _nc_cache["nc"]

    D = _dct_mat()
    A = np.kron(np.eye(16, dtype=np.float32), D).astype(np.float32)
    I64 = np.eye(64, dtype=np.float32)
    cW = np.stack(
        [np.vstack([conv_w[:, :, d].T, conv_w[:, :, d].T]) for d in range(3)]
    ).astype(np.float32)  # (3, 128, 64): [d][ci(dup), co]
    consts = {
        "cAT": np.ascontiguousarray(A.T),
        "cA": np.ascontiguousarray(A),
        "cI128": np.eye(128, dtype=np.float32),
        "cI64d": np.ascontiguousarray(np.vstack([I64, I64])),
        "cW": np.ascontiguousarray(cW),
        "cBd": np.ascontiguousarray(
            np.concatenate([conv_b, conv_b]).reshape(128, 1)),
    }
    in_maps = [{"x": np.ascontiguousarray(x[i]), **consts} for i in range(bsz)]
    res = bass_utils.run_bass_kernel_spmd(nc, in_maps, core_ids=list(range(N_CORES)))
    out = np.stack([res.results[i]["out"] for i in range(bsz)])
    return out.astype(np.float32)
